# revision 101
# baseline (speedup 1.0000x reference)
"""Trainium2 Bass kernel for HeavilyCompressedAttention.

Sharding: 16 heads across 8 cores (2 heads/core, tensor-parallel);
compressed-KV path (single shared head) replicated on every core;
out_proj row-parallel with host-side partial sum (bf16 partials).

Per-core device pipeline (all matmuls bf16, fp32 accumulation):
  P1: q/lk/lv/compress-score projections from a fully SBUF-resident
      hidden^T (16 big row DMAs), fused RMSNorm + partial RoPE, then
      PE transposes into qT/lkT.
  P2: learned-weighted KV compression (entries) + shared ck/cv head.
  P3: attention with TRANSPOSED scores [t, s] so the exp writes the
      transposed probabilities directly (no per-prob transposes);
      denominators fused into the ctx matmuls via an appended
      ones-column; masks are post-exp binary multiplies on GpSimd.
  P4: out-projection (row-parallel partial, 0.5 folded into Wo),
      bf16 stores.
"""

import os
import sys

import numpy as np
import ml_dtypes

for _p in ("/opt/trn_rl_repo", "/root/.axon_site/_ro/trn_rl_repo"):
    if os.path.isdir(_p) and _p not in sys.path:
        sys.path.insert(0, _p)

from concourse import bacc, mybir  # noqa: E402
import concourse.tile as tile  # noqa: E402
from concourse.bass_utils import run_bass_kernel_spmd  # noqa: E402
from concourse.masks import make_identity  # noqa: E402

F32 = mybir.dt.float32
BF16 = mybir.dt.bfloat16
FP8 = mybir.dt.float8e4
NPBF = ml_dtypes.bfloat16
NPF8 = ml_dtypes.float8_e4m3
WO_SCALE = 64.0
ALU = mybir.AluOpType

S = 2048
HID = 2048
NH = 16
HD = 128
R = 16
C = S // R  # 128
WIN = 128
ROPE = HD // 2  # 64
HALF = ROPE // 2  # 32
EPS = 1e-6
NT = S // 128  # 16 s-tiles
KT = HID // 128  # 16 k-tiles
NCORES = 8
HPC = NH // NCORES  # 2 heads per core
SCALE = 1.0 / float(np.sqrt(HD))

_CACHE = {}


def _build_bass(debug=False):
    nc = bacc.Bacc("TRN2", target_bir_lowering=False, debug=False,
                   num_devices=NCORES)

    din = {}

    def inp(name, shape, dt):
        din[name] = nc.dram_tensor(name, list(shape), dt, kind="ExternalInput")
        return din[name]

    hT = inp("hT", [KT, 128, S], BF16)            # hidden^T k-rows
    hN = inp("hN", [NT, 128, HID], BF16)          # hidden natural s-tiles
    wqlk = inp("wqlk", [128, KT, 512], BF16)      # [q0|q1|lk0|lk1] col-slices
    wlvc = inp("wlvc", [128, KT, 257], BF16)      # [lv0|lv1|Wc]
    wkv = inp("wkv", [128, KT, 256], BF16)        # [Wk|Wv] shared head
    wo = inp("wo", [128, HPC, HID], BF16)         # 0.5*Wo rows per head
    b_qlk = inp("b_qlk", [1, 512], BF16)
    b_lvc = inp("b_lvc", [1, 257], BF16)
    b_kv = inp("b_kv", [1, 256], BF16)
    tcos = inp("tcos", [128, NT, HALF], F32)      # rope cos table
    tsin = inp("tsin", [128, NT, HALF], F32)      # rope sin table
    wqk = inp("wqk", [128, 512], BF16)            # [qn_w|qn_w|kn_w|kn_w] rows
    ctA = inp("ctA", [C, HALF], F32)              # ck rope tables (block_ends)
    ctB = inp("ctB", [C, HALF], F32)
    ctC = inp("ctC", [C, HALF], F32)
    ctD = inp("ctD", [C, HALF], F32)
    ck_pass = inp("ck_pass", [C, ROPE], F32)      # kn_w[64:] bcast rows
    mbinL1 = inp("mbinL1", [128, 512], FP8)       # additive local mask [P,P,C,C]
    mbinK2 = inp("mbinK2", [128, NT, 256], FP8)   # comp mask [c, i, 2x128]
    ident8 = inp("ident8", [128, 128], FP8)       # fp8 identity
    dwide = inp("dwide", [128, 248], BF16)        # sliding block-diag pattern
    sinkkT = inp("sinkkT", [128, HPC], BF16)      # sink_k columns [d, h]
    sinkvo = inp("sinkvo", [1, HPC, 129], BF16)   # rows [sink_v[h] | 1.0]

    out_p = nc.dram_tensor("out_p", [S, HID], BF16, kind="ExternalOutput")
    dbg = {}
    if debug:
        for name, shape in [("qTb", [128, NT, 256]), ("lkTb", [128, NT, 256]),
                            ("lvo", [128, NT, HPC, 129]), ("cvo", [C, 129]),
                            ("ckT", [128, C]), ("wN", [128, NT]),
                            ("eT", [128, KT, C]),
                            ("pt5", [128, 768]), ("mg5", [128, 256]),
                            ("pstS5", [1, 256])]:
            dbg[name] = nc.dram_tensor("dbg_" + name, shape, BF16,
                                       kind="ExternalOutput")
        dbg["mgTb"] = nc.dram_tensor("dbg_mgTb", [128, HPC, S], BF16,
                                     kind="ExternalOutput")

    with tile.TileContext(nc) as tc:
        with (
            tc.tile_pool(name="const", bufs=1) as cst,
            tc.tile_pool(name="persist", bufs=1) as per,
            tc.tile_pool(name="stream", bufs=6) as stm,
            tc.tile_pool(name="scratch", bufs=3) as scr,
            tc.tile_pool(name="stats", bufs=6) as sts,
        ):
            def load(name, shape, dt):
                t = cst.tile(list(shape), dt, name=f"c_{name}")
                nc.sync.dma_start(out=t[:], in_=din[name].ap())
                return t

            # ---- weight/table loads, interleaved so P1's k-stream starts
            # immediately: each k-group (wqlk_k, wlvc_k, hT_k) arrives ~2us
            # apart and tile i=0..2 matmuls consume them as they land.
            wqlk_sb = cst.tile([128, KT, 512], BF16, name="c_wqlk")
            wlvc_sb = cst.tile([128, KT, 257], BF16, name="c_wlvc")
            hT_sb = cst.tile([128, KT, S], BF16, name="c_hT")
            nc.sync.dma_start(out=wqlk_sb[:, 0, :], in_=wqlk.ap()[:, 0, :])
            nc.sync.dma_start(out=wlvc_sb[:, 0, :], in_=wlvc.ap()[:, 0, :])
            nc.sync.dma_start(out=hT_sb[:, 0, :], in_=hT.ap()[0])
            b_qlk_sb = load("b_qlk", [1, 512], BF16)
            b_lvc_sb = load("b_lvc", [1, 257], BF16)
            tcos_sb = load("tcos", [128, NT, HALF], F32)
            tsin_sb = load("tsin", [128, NT, HALF], F32)
            wqk_sb = load("wqk", [128, 512], BF16)
            for k in range(1, KT):
                nc.sync.dma_start(out=wqlk_sb[:, k, :], in_=wqlk.ap()[:, k, :])
                nc.sync.dma_start(out=wlvc_sb[:, k, :], in_=wlvc.ap()[:, k, :])
                nc.sync.dma_start(out=hT_sb[:, k, :], in_=hT.ap()[k])
            wkv_sb = load("wkv", [128, KT, 256], BF16)
            b_kv_sb = load("b_kv", [1, 256], BF16)
            ctA_sb = load("ctA", [C, HALF], F32)
            ctB_sb = load("ctB", [C, HALF], F32)
            ctC_sb = load("ctC", [C, HALF], F32)
            ctD_sb = load("ctD", [C, HALF], F32)
            ck_pass_sb = load("ck_pass", [C, ROPE], F32)
            mbinL1_sb = load("mbinL1", [128, 512], FP8)
            mbinK2_sb = load("mbinK2", [128, NT, 256], FP8)
            ident8_sb = load("ident8", [128, 128], FP8)
            dwide_sb = load("dwide", [128, 248], BF16)
            sinkkT_sb = load("sinkkT", [128, HPC], BF16)
            sinkvo_sb = load("sinkvo", [1, HPC, 129], BF16)
            wo_sb = load("wo", [128, HPC, HID], BF16)

            ident_bf = cst.tile([128, 128], BF16)
            make_identity(nc, ident_bf[:])
            ident_f32 = cst.tile([128, 128], F32)
            make_identity(nc, ident_f32[:])
            ones1 = cst.tile([1, 128], BF16)
            nc.vector.memset(ones1[:], 1.0)
            eps_t = cst.tile([128, 1], F32)
            nc.vector.memset(eps_t[:], EPS)

            # ---- persistent activations ----
            qTb = per.tile([128, NT, 256], BF16)     # q^T per head
            lkTb = per.tile([128, NT, 256], BF16)    # lk^T per head
            lvo = per.tile([128, NT, HPC, 129], BF16)  # lv | ones col
            cvo = per.tile([C, 129], BF16)           # cv | ones col
            ckT = per.tile([128, C], BF16)           # ck^T (shared head)
            mgTb = per.tile([128, HPC, S], BF16)     # merged^T
            cwN = per.tile([128, NT], F32)           # compressor scores
            wN = per.tile([128, NT], F32)            # block-softmaxed weights
            eT = per.tile([128, KT, C], BF16)        # entries^T tiles

            nc.vector.memset(lvo[:, :, :, 128:129], 1.0)
            nc.vector.memset(cvo[:, 128:129], 1.0)

            # ================= P1: projections + norm/rope =================
            with (
                tc.tile_pool(name="ps_qlk", bufs=4, space="PSUM") as pq,
                tc.tile_pool(name="ps_lvc", bufs=4, space="PSUM") as pl,
            ):
                NFILL = 4
                fill_q, fill_l = [], []
                for i in range(NFILL):
                    fq = pq.tile([128, 512], F32, tag="ps_q", name="ps_q")
                    fill_q.append(fq)
                    fl = pl.tile([128, 257], F32, tag="ps_l", name="ps_l")
                    fill_l.append(fl)
                # k-major for the first NFILL tiles: PE consumes each hT row
                # as it arrives from HBM instead of stalling on tile 0
                for k in range(KT):
                    for i in range(NFILL):
                        hk = hT_sb[:, k, i * 128:(i + 1) * 128]
                        nc.tensor.matmul(fill_q[i][:], hk, wqlk_sb[:, k, :],
                                         start=(k == 0), stop=False)
                        nc.tensor.matmul(fill_l[i][:], hk, wlvc_sb[:, k, :],
                                         start=(k == 0), stop=False)
                for i in range(NT):
                    if i < NFILL:
                        ps_q, ps_l = fill_q[i], fill_l[i]
                    else:
                        ps_q = pq.tile([128, 512], F32, tag="ps_q")
                        ps_l = pl.tile([128, 257], F32, tag="ps_l")
                        for k in range(KT):
                            hk = hT_sb[:, k, i * 128:(i + 1) * 128]
                            nc.tensor.matmul(ps_q[:], hk, wqlk_sb[:, k, :],
                                             start=(k == 0), stop=False)
                            nc.tensor.matmul(ps_l[:], hk, wlvc_sb[:, k, :],
                                             start=(k == 0), stop=False)
                    nc.tensor.matmul(ps_q[:], ones1[:], b_qlk_sb[:],
                                     start=False, stop=True)
                    nc.tensor.matmul(ps_l[:], ones1[:], b_lvc_sb[:],
                                     start=False, stop=True)

                    # rms norm (over d) for the 4 sub-tensors [q0|q1|k0|k1]
                    ssq = sts.tile([128, 4], F32)
                    for j in range(4):
                        sq_s = scr.tile([128, 128], F32, tag="sq", bufs=2)
                        nc.scalar.activation(
                            sq_s[:], ps_q[:, j * 128:(j + 1) * 128],
                            mybir.ActivationFunctionType.Square,
                            accum_out=ssq[:, j:j + 1])
                    rms = sts.tile([128, 4], F32)
                    nc.scalar.activation(rms[:], ssq[:],
                                         mybir.ActivationFunctionType.Sqrt,
                                         scale=1.0 / HD, bias=eps_t[:])
                    rinv = sts.tile([128, 4], F32)
                    nc.vector.reciprocal(rinv[:], rms[:])

                    qn = scr.tile([128, 512], BF16, tag="qn", bufs=4)
                    nc.vector.tensor_mul(
                        qn[:].rearrange("p (a b) -> p a b", a=4),
                        ps_q[:].rearrange("p (a b) -> p a b", a=4),
                        rinv[:].unsqueeze(2).broadcast_to([128, 4, 128]))
                    nc.vector.tensor_mul(qn[:], qn[:], wqk_sb[:])
                    # partial rope on cols [0:64) of each sub-tensor
                    qn4 = qn[:].rearrange("p (s r b) -> p s r b", s=2, r=2)
                    x1 = qn4[:, :, :, 0:HALF]
                    x2 = qn4[:, :, :, HALF:ROPE]

                    def tslice(t):
                        return (t[:, i, :].unsqueeze(1).unsqueeze(1)
                                .broadcast_to([128, 2, 2, HALF]))

                    t1 = scr.tile([128, 4, HALF], BF16, tag="t1", bufs=3)
                    t2 = scr.tile([128, 4, HALF], BF16, tag="t2", bufs=3)
                    t3 = scr.tile([128, 4, HALF], BF16, tag="t3", bufs=3)
                    t4 = scr.tile([128, 4, HALF], BF16, tag="t4", bufs=3)

                    def v4(t):
                        return t[:].rearrange("p (s r) c -> p s r c", s=2)

                    nc.vector.tensor_mul(v4(t1), x1, tslice(tcos_sb))
                    nc.vector.tensor_mul(v4(t2), x2, tslice(tsin_sb))
                    nc.vector.tensor_mul(v4(t3), x1, tslice(tsin_sb))
                    nc.vector.tensor_mul(v4(t4), x2, tslice(tcos_sb))
                    nc.vector.tensor_sub(x1, v4(t1), v4(t2))
                    nc.vector.tensor_add(x2, v4(t3), v4(t4))

                    # collect compressor scores + local V (with ones col)
                    nc.scalar.copy(cwN[:, i:i + 1], ps_l[:, 256:257])
                    nc.scalar.copy(
                        lvo[:, i, :, 0:128],
                        ps_l[:, 0:256].rearrange("p (h d) -> p h d", h=2))

                    # transposes into qT / lkT via DMA xbar (HWDGE is idle
                    # during P1; results not needed until P3)
                    for j in range(4):
                        dst = (qTb[:, i, j * 128:(j + 1) * 128] if j < 2 else
                               lkTb[:, i, (j - 2) * 128:(j - 1) * 128])
                        nc.sync.dma_start_transpose(
                            dst, qn[:, j * 128:(j + 1) * 128])

            # ================= P2: entries + ck/cv =================
            with (
                tc.tile_pool(name="ps_e", bufs=1, space="PSUM") as pe,
                tc.tile_pool(name="ps_kv", bufs=1, space="PSUM") as pkv,
                tc.tile_pool(name="ps_tp2", bufs=2, space="PSUM") as ptp2,
            ):
                # block-softmax of compressor scores (R=16 blocks)
                ps_cw1 = ptp2.tile([16, 128], F32, tag="tp2")
                nc.tensor.transpose(ps_cw1[:], cwN[:], ident_f32[:])
                cwT = scr.tile([16, 128], F32, tag="cwT", bufs=1)
                nc.scalar.copy(cwT[:], ps_cw1[:])
                cw3 = cwT[:].rearrange("p (g r) -> p g r", g=8)
                cmx = sts.tile([16, 8], F32)
                nc.vector.tensor_reduce(cmx[:], cw3, mybir.AxisListType.X,
                                        ALU.max)
                cwE = scr.tile([16, 128], F32, tag="cwE", bufs=1)
                nc.vector.tensor_sub(
                    cwE[:].rearrange("p (g r) -> p g r", g=8), cw3,
                    cmx[:].unsqueeze(2).broadcast_to([16, 8, 16]))
                nc.scalar.activation(cwE[:], cwE[:],
                                     mybir.ActivationFunctionType.Exp)
                csum = sts.tile([16, 8], F32)
                nc.vector.tensor_reduce(
                    csum[:], cwE[:].rearrange("p (g r) -> p g r", g=8),
                    mybir.AxisListType.X, ALU.add)
                crec = sts.tile([16, 8], F32)
                nc.vector.reciprocal(crec[:], csum[:])
                cwW = scr.tile([16, 128], F32, tag="cwW", bufs=1)
                nc.vector.tensor_mul(
                    cwW[:].rearrange("p (g r) -> p g r", g=8),
                    cwE[:].rearrange("p (g r) -> p g r", g=8),
                    crec[:].unsqueeze(2).broadcast_to([16, 8, 16]))
                ps_cw2 = ptp2.tile([128, 16], F32, tag="tp2")
                nc.tensor.transpose(ps_cw2[:], cwW[:], ident_f32[0:16, 0:16])
                nc.scalar.copy(wN[:], ps_cw2[:])

                # entries natural [C, HID] (4 bank-aligned accum groups),
                # then transpose into eT
                ps_e = pe.tile([C, HID], F32)
                entries = scr.tile([C, HID], BF16, tag="entries", bufs=1)
                for i in range(NT):
                    wbig = scr.tile([128, 128], BF16, tag="wbig", bufs=2)
                    nc.vector.tensor_scalar_mul(
                        wbig[:], dwide_sb[:, 120 - 8 * i:248 - 8 * i],
                        wN[:, i:i + 1])
                    hN_t = stm.tile([128, HID], BF16, tag="hN", bufs=5)
                    nc.sync.dma_start(out=hN_t[:], in_=hN.ap()[i])
                    for hc in range(4):
                        nc.tensor.matmul(ps_e[:, hc * 512:(hc + 1) * 512],
                                         wbig[:],
                                         hN_t[:, hc * 512:(hc + 1) * 512],
                                         start=(i == 0), stop=(i == NT - 1))
                for hc in range(4):
                    if hc % 2 == 0:
                        nc.vector.tensor_copy(entries[:, hc * 512:(hc + 1) * 512],
                                              ps_e[:, hc * 512:(hc + 1) * 512])
                    else:
                        nc.scalar.copy(entries[:, hc * 512:(hc + 1) * 512],
                                       ps_e[:, hc * 512:(hc + 1) * 512])
                for k in range(KT):
                    ps_t = ptp2.tile([128, 128], BF16, tag="tp2")
                    nc.tensor.transpose(ps_t[:],
                                        entries[:, k * 128:(k + 1) * 128],
                                        ident_bf[:])
                    if k % 2 == 0:
                        nc.vector.tensor_copy(eT[:, k, :], ps_t[:])
                    else:
                        nc.scalar.copy(eT[:, k, :], ps_t[:])

                ps_kv = pkv.tile([C, 256], F32)
                for k in range(KT):
                    nc.tensor.matmul(ps_kv[:], eT[:, k, :], wkv_sb[:, k, :],
                                     start=(k == 0), stop=False)
                nc.tensor.matmul(ps_kv[:], ones1[:], b_kv_sb[:],
                                 start=False, stop=True)

                # ck: rmsnorm + rope at block-end positions
                ssqc = sts.tile([C, 1], F32)
                sq_c = scr.tile([C, 128], F32, tag="sq", bufs=2)
                nc.scalar.activation(sq_c[:], ps_kv[:, 0:128],
                                     mybir.ActivationFunctionType.Square,
                                     accum_out=ssqc[:])
                rmsc = sts.tile([C, 1], F32)
                nc.scalar.activation(rmsc[:], ssqc[:],
                                     mybir.ActivationFunctionType.Sqrt,
                                     scale=1.0 / HD, bias=eps_t[:])
                rinvc = sts.tile([C, 1], F32)
                nc.vector.reciprocal(rinvc[:], rmsc[:])
                ckn = scr.tile([C, 128], F32, tag="ckn", bufs=1)
                nc.vector.tensor_scalar_mul(ckn[:], ps_kv[:, 0:128], rinvc[:])
                ckR = scr.tile([C, 128], BF16, tag="ckR", bufs=1)
                ct1 = scr.tile([C, HALF], F32, tag="ct1", bufs=1)
                ct2 = scr.tile([C, HALF], F32, tag="ct2", bufs=1)
                nc.vector.tensor_mul(ct1[:], ckn[:, 0:HALF], ctA_sb[:])
                nc.vector.tensor_mul(ct2[:], ckn[:, HALF:ROPE], ctB_sb[:])
                nc.vector.tensor_sub(ckR[:, 0:HALF], ct1[:], ct2[:])
                nc.vector.tensor_mul(ct1[:], ckn[:, 0:HALF], ctC_sb[:])
                nc.vector.tensor_mul(ct2[:], ckn[:, HALF:ROPE], ctD_sb[:])
                nc.vector.tensor_add(ckR[:, HALF:ROPE], ct1[:], ct2[:])
                nc.vector.tensor_mul(ckR[:, ROPE:128], ckn[:, ROPE:128],
                                     ck_pass_sb[:])
                nc.scalar.copy(cvo[:, 0:128], ps_kv[:, 128:256])
                ps_ct = ptp2.tile([128, 128], BF16, tag="tp2")
                nc.tensor.transpose(ps_ct[:], ckR[:], ident_bf[:])
                nc.scalar.copy(ckT[:], ps_ct[:])

            # ================= P3: attention (transposed scores) ===========
            with (
                tc.tile_pool(name="ps_sc", bufs=1, space="PSUM") as psc,
                tc.tile_pool(name="ps_ctx", bufs=1, space="PSUM") as pcx,
                tc.tile_pool(name="ps_snk", bufs=1, space="PSUM") as psnk,
                tc.tile_pool(name="ps_mt", bufs=1, space="PSUM") as pmt,
                tc.tile_pool(name="ps_out", bufs=2, space="PSUM") as pout,
            ):
                for i in range(NT):
                    qT_i = qTb[:, i, :]
                    # sink scores^T, heads side-by-side: [1, h*128+s]
                    ps_sink = psnk.tile([1, HPC * 128], F32)
                    for h in range(HPC):
                        nc.tensor.matmul(ps_sink[:, h * 128:(h + 1) * 128],
                                         sinkkT_sb[:, h:h + 1],
                                         qT_i[:, h * 128:(h + 1) * 128],
                                         start=True, stop=True)
                    pstS = scr.tile([1, HPC * 128], BF16, tag="pstS")
                    nc.scalar.activation(pstS[:], ps_sink[:],
                                         mybir.ActivationFunctionType.Exp,
                                         scale=SCALE)

                    # one transposed score tile [prev-h0,prev-h1,cur-h0,
                    # cur-h1,comp-h0,comp-h1], one exp, masks split
                    # Pool (local) / DVE (comp)
                    ps_s = psc.tile([128, 768], F32, tag="sc")
                    # one mask-add matmul per PSUM bank (single start=True),
                    # then score matmuls accumulate (WAW-ordered after mask)
                    nc.tensor.matmul(ps_s[:, 0:512], ident8_sb[:],
                                     mbinL1_sb[:], start=True, stop=False,
                                     skip_group_check=True)
                    nc.tensor.matmul(ps_s[:, 512:768], ident8_sb[:],
                                     mbinK2_sb[:, i, :], start=True,
                                     stop=False, skip_group_check=True)
                    for h in range(HPC):
                        qh = qT_i[:, h * 128:(h + 1) * 128]
                        if i > 0:
                            nc.tensor.matmul(
                                ps_s[:, h * 128:(h + 1) * 128],
                                lkTb[:, i - 1, h * 128:(h + 1) * 128], qh,
                                start=False, stop=True, skip_group_check=True)
                        nc.tensor.matmul(
                            ps_s[:, 256 + h * 128:256 + (h + 1) * 128],
                            lkTb[:, i, h * 128:(h + 1) * 128], qh,
                            start=False, stop=True, skip_group_check=True)
                        nc.tensor.matmul(
                            ps_s[:, 512 + h * 128:512 + (h + 1) * 128],
                            ckT[:], qh, start=False, stop=True,
                            skip_group_check=True)
                    if i == 0:
                        nc.vector.memset(ps_s[:, 0:256], -30000.0)
                    pt = scr.tile([128, 768], BF16, tag="pt", bufs=4)
                    nc.scalar.activation(pt[:], ps_s[:],
                                         mybir.ActivationFunctionType.Exp,
                                         scale=SCALE)

                    # ctx accumulation: [ctx | den] via appended ones column
                    ps_l3 = pcx.tile([128, HPC * 129], F32, tag="ctxl")
                    ps_c3 = pcx.tile([128, HPC * 129], F32, tag="ctxc")
                    for h in range(HPC):
                        o = h * 129
                        if i > 0:
                            nc.tensor.matmul(ps_l3[:, o:o + 129],
                                             pt[:, h * 128:(h + 1) * 128],
                                             lvo[:, i - 1, h, :],
                                             start=True, stop=False)
                        nc.tensor.matmul(ps_l3[:, o:o + 129],
                                         pt[:, 256 + h * 128:256 + (h + 1) * 128],
                                         lvo[:, i, h, :],
                                         start=(i == 0), stop=True)
                        nc.tensor.matmul(ps_c3[:, o:o + 129],
                                         pt[:, 512 + h * 128:512 + (h + 1) * 128],
                                         cvo[:], start=True, stop=False)
                        nc.tensor.matmul(ps_c3[:, o:o + 129],
                                         pstS[:, h * 128:(h + 1) * 128],
                                         sinkvo_sb[:, h, :],
                                         start=False, stop=True)

                    # normalize + combine: mg = ctxl*rdl + ctxc*rdc
                    rd = sts.tile([128, 4], F32)
                    nc.vector.reciprocal(
                        rd[:, 0:2].unsqueeze(2),
                        ps_l3[:].rearrange("p (h x) -> p h x", h=2)[:, :, 128:129])
                    nc.vector.reciprocal(
                        rd[:, 2:4].unsqueeze(2),
                        ps_c3[:].rearrange("p (h x) -> p h x", h=2)[:, :, 128:129])
                    mg = scr.tile([128, 256], BF16, tag="mg", bufs=2)
                    for h in range(HPC):
                        o = h * 129
                        tmp = scr.tile([128, 128], F32, tag="cmb", bufs=2)
                        nc.vector.tensor_scalar_mul(
                            tmp[:], ps_c3[:, o:o + 128], rd[:, 2 + h:3 + h])
                        nc.vector.scalar_tensor_tensor(
                            mg[:, h * 128:(h + 1) * 128],
                            ps_l3[:, o:o + 128], rd[:, h:h + 1],
                            tmp[:], ALU.mult, ALU.add)
                    if debug and i == 5:
                        nc.sync.dma_start(out=dbg["pt5"].ap(), in_=pt[:])
                        nc.sync.dma_start(out=dbg["mg5"].ap(), in_=mg[:])
                        nc.sync.dma_start(out=dbg["pstS5"].ap(), in_=pstS[:])
                    for h in range(HPC):
                        ps_mt = pmt.tile([128, 128], BF16, tag="mt")
                        nc.tensor.transpose(
                            ps_mt[:], mg[:, h * 128:(h + 1) * 128], ident_bf[:])
                        nc.vector.tensor_copy(
                            mgTb[:, h, i * 128:(i + 1) * 128], ps_mt[:])

                    # out projection, interleaved per tile
                    for oc in range(4):
                        ps_oc = pout.tile([128, 512], F32, tag="oc")
                        for h in range(HPC):
                            nc.tensor.matmul(
                                ps_oc[:], mgTb[:, h, i * 128:(i + 1) * 128],
                                wo_sb[:, h, oc * 512:(oc + 1) * 512],
                                start=(h == 0), stop=(h == HPC - 1))
                        o_sb = scr.tile([128, 512], BF16,
                                        tag=f"osb{oc % 2}", bufs=2)
                        if oc < 2:
                            nc.scalar.copy(o_sb[:], ps_oc[:])
                        else:
                            nc.vector.tensor_copy(o_sb[:], ps_oc[:])
                        nc.sync.dma_start(
                            out=out_p.ap()[i * 128:(i + 1) * 128,
                                           oc * 512:(oc + 1) * 512],
                            in_=o_sb[:])
            if debug:
                for name, t in [("qTb", qTb), ("lkTb", lkTb), ("lvo", lvo),
                                ("cvo", cvo), ("ckT", ckT), ("eT", eT),
                                ("mgTb", mgTb)]:
                    nc.sync.dma_start(out=dbg[name].ap(), in_=t[:])
                wNb = per.tile([128, NT], BF16)
                nc.vector.tensor_copy(wNb[:], wN[:])
                nc.sync.dma_start(out=dbg["wN"].ap(), in_=wNb[:])

    nc.compile()
    return nc


def _host_prep(inputs):
    """Build the 8 per-core input maps from full inputs."""
    hs = np.asarray(inputs["hidden_states"], np.float32)[0]  # [S, HID]
    Wq = np.asarray(inputs["Wq"], np.float32)
    Wc = np.asarray(inputs["Wc"], np.float32)
    Wk = np.asarray(inputs["Wk"], np.float32)
    Wv = np.asarray(inputs["Wv"], np.float32)
    Wlk = np.asarray(inputs["Wlk"], np.float32)
    Wlv = np.asarray(inputs["Wlv"], np.float32)
    qn_w = np.asarray(inputs["qn_w"], np.float32)
    kn_w = np.asarray(inputs["kn_w"], np.float32)
    sink_k = np.asarray(inputs["sink_k"], np.float32)
    sink_v = np.asarray(inputs["sink_v"], np.float32)
    Wo = np.asarray(inputs["Wo"], np.float32)
    bq = np.asarray(inputs["bq"], np.float32)
    bc = np.asarray(inputs["bc"], np.float32)
    bk = np.asarray(inputs["bk"], np.float32)
    bv = np.asarray(inputs["bv"], np.float32)
    blk = np.asarray(inputs["blk"], np.float32)
    blv = np.asarray(inputs["blv"], np.float32)

    hT_t = np.ascontiguousarray(hs.T).reshape(KT, 128, S).astype(NPBF)
    hN_t = hs.reshape(NT, 128, HID).astype(NPBF)

    def dev_w(w):  # [HID, F] -> [128, KT, F]
        return np.ascontiguousarray(
            w.reshape(KT, 128, -1).transpose(1, 0, 2)).astype(NPBF)

    # rope tables for token positions (q & k variants, norm weight folded)
    pos = np.arange(S, dtype=np.float32)
    inv_freq = 1.0 / (10000.0 ** (np.arange(HALF, dtype=np.float32) * 2.0 / ROPE))
    ang = pos[:, None] * inv_freq[None, :]
    cos, sin = np.cos(ang), np.sin(ang)  # [S, HALF]

    def pack(t):  # [S,HALF] -> [128, NT, HALF]
        return np.ascontiguousarray(
            t.reshape(NT, 128, HALF).transpose(1, 0, 2)).astype(np.float32)

    kw1, kw2 = kn_w[0:HALF], kn_w[HALF:ROPE]
    tcos = pack(cos)
    tsin = pack(sin)
    wqk = np.broadcast_to(
        np.concatenate([qn_w, qn_w, kn_w, kn_w])[None, :],
        (128, 512)).astype(NPBF).copy()

    # ck rope tables at block-end positions
    pos_c = (np.arange(C, dtype=np.float32) * R + (R - 1))
    angc = pos_c[:, None] * inv_freq[None, :]
    cosc, sinc = np.cos(angc), np.sin(angc)
    ctA = (cosc * kw1[None, :]).astype(np.float32)
    ctB = (sinc * kw2[None, :]).astype(np.float32)
    ctC = (sinc * kw1[None, :]).astype(np.float32)
    ctD = (cosc * kw2[None, :]).astype(np.float32)
    ck_pass = np.broadcast_to(kn_w[ROPE:][None, :],
                              (C, ROPE)).astype(np.float32).copy()

    # binary masks (transposed score layout [t, s] / [c, s])
    tl = np.arange(128)[:, None]
    sl = np.arange(128)[None, :]
    mbinP = np.where(tl >= sl, 0.0, -240.0).astype(np.float32)
    mbinC = np.where(tl <= sl, 0.0, -240.0).astype(np.float32)
    mbinL1 = np.concatenate([mbinP, mbinP, mbinC, mbinC], axis=1).astype(NPF8)
    cc = np.arange(128)[:, None]
    sg = np.arange(S)[None, :]
    mK = np.where(cc * R + (R - 1) <= sg, 0.0, -240.0).astype(np.float32)
    mK = mK.reshape(128, NT, 128)
    mbinK2 = np.concatenate([mK, mK], axis=2).astype(NPF8)
    ident8 = np.eye(128, dtype=np.float32).astype(NPF8)
    idx2 = np.arange(248)[None, :]
    dwide = (idx2 == 120 + np.arange(128)[:, None] // 16).astype(
        np.float32).astype(NPBF)

    common = dict(hT=hT_t, hN=hN_t, tcos=tcos, tsin=tsin, wqk=wqk,
                  ctA=ctA, ctB=ctB, ctC=ctC, ctD=ctD,
                  ck_pass=ck_pass, mbinL1=mbinL1, mbinK2=mbinK2, ident8=ident8,
                  dwide=dwide,
                  wkv=dev_w(np.concatenate([Wk, Wv], axis=1)),
                  b_kv=np.concatenate([bk, bv])[None, :].astype(NPBF))

    Wq4 = Wq.reshape(HID, NH, HD)
    Wlk4 = Wlk.reshape(HID, NH, HD)
    Wlv4 = Wlv.reshape(HID, NH, HD)
    bq4 = bq.reshape(NH, HD)
    blk4 = blk.reshape(NH, HD)
    blv4 = blv.reshape(NH, HD)
    Wo4 = Wo.reshape(NH, HD, HID)

    in_maps = []
    for c in range(NCORES):
        hh = [HPC * c + h for h in range(HPC)]
        wqlk = np.concatenate([Wq4[:, hh[0]], Wq4[:, hh[1]],
                               Wlk4[:, hh[0]], Wlk4[:, hh[1]]], axis=1)
        wlvc = np.concatenate([Wlv4[:, hh[0]], Wlv4[:, hh[1]], Wc], axis=1)
        b_qlk = np.concatenate([bq4[hh[0]], bq4[hh[1]],
                                blk4[hh[0]], blk4[hh[1]]])[None, :]
        b_lvc = np.concatenate([blv4[hh[0]], blv4[hh[1]], bc])[None, :]
        wo_c = np.ascontiguousarray(
            (0.5 * Wo4[hh]).transpose(1, 0, 2)).astype(NPBF)
        sinkkT = np.ascontiguousarray(sink_k[hh].T).astype(NPBF)  # [128, HPC]
        sinkvo = np.zeros((1, HPC, 129), np.float32)
        for h in range(HPC):
            sinkvo[0, h, 0:128] = sink_v[hh[h]]
            sinkvo[0, h, 128] = 1.0
        m = dict(common)
        m.update(wqlk=dev_w(wqlk), wlvc=dev_w(wlvc),
                 b_qlk=b_qlk.astype(NPBF), b_lvc=b_lvc.astype(NPBF),
                 wo=wo_c, sinkkT=sinkkT, sinkvo=sinkvo.astype(NPBF))
        in_maps.append(m)
    return in_maps


def kernel(**inputs):
    if "nc" not in _CACHE:
        _CACHE["nc"] = _build_bass()
    nc = _CACHE["nc"]
    in_maps = _host_prep(inputs)
    res = run_bass_kernel_spmd(nc, in_maps, core_ids=list(range(NCORES)))
    out = np.zeros((S, HID), np.float64)
    for c in range(NCORES):
        out += res.results[c]["out_p"].astype(np.float64)
    out += np.asarray(inputs["bo"], np.float32)[None, :]
    return out[None].astype(np.float32)


# revision 102
# speedup vs baseline: 1.0004x; 1.0004x over previous
"""Trainium2 Bass kernel for HeavilyCompressedAttention.

Sharding: 16 heads across 8 cores (2 heads/core, tensor-parallel);
compressed-KV path (single shared head) replicated on every core;
out_proj row-parallel with host-side partial sum (bf16 partials).

Per-core device pipeline (all matmuls bf16, fp32 accumulation):
  P1: q/lk/lv/compress-score projections from a fully SBUF-resident
      hidden^T (16 big row DMAs), fused RMSNorm + partial RoPE, then
      PE transposes into qT/lkT.
  P2: learned-weighted KV compression (entries) + shared ck/cv head.
  P3: attention with TRANSPOSED scores [t, s] so the exp writes the
      transposed probabilities directly (no per-prob transposes);
      denominators fused into the ctx matmuls via an appended
      ones-column; masks are post-exp binary multiplies on GpSimd.
  P4: out-projection (row-parallel partial, 0.5 folded into Wo),
      bf16 stores.
"""

import os
import sys

import numpy as np
import ml_dtypes

for _p in ("/opt/trn_rl_repo", "/root/.axon_site/_ro/trn_rl_repo"):
    if os.path.isdir(_p) and _p not in sys.path:
        sys.path.insert(0, _p)

from concourse import bacc, mybir  # noqa: E402
import concourse.tile as tile  # noqa: E402
from concourse.bass_utils import run_bass_kernel_spmd  # noqa: E402
from concourse.masks import make_identity  # noqa: E402

F32 = mybir.dt.float32
BF16 = mybir.dt.bfloat16
FP8 = mybir.dt.float8e4
NPBF = ml_dtypes.bfloat16
NPF8 = ml_dtypes.float8_e4m3
WO_SCALE = 64.0
ALU = mybir.AluOpType

S = 2048
HID = 2048
NH = 16
HD = 128
R = 16
C = S // R  # 128
WIN = 128
ROPE = HD // 2  # 64
HALF = ROPE // 2  # 32
EPS = 1e-6
NT = S // 128  # 16 s-tiles
KT = HID // 128  # 16 k-tiles
NCORES = 8
HPC = NH // NCORES  # 2 heads per core
SCALE = 1.0 / float(np.sqrt(HD))

_CACHE = {}


def _build_bass(debug=False):
    nc = bacc.Bacc("TRN2", target_bir_lowering=False, debug=False,
                   num_devices=NCORES)

    din = {}

    def inp(name, shape, dt):
        din[name] = nc.dram_tensor(name, list(shape), dt, kind="ExternalInput")
        return din[name]

    hT = inp("hT", [KT, 128, S], BF16)            # hidden^T k-rows
    hN = inp("hN", [NT, 128, HID], BF16)          # hidden natural s-tiles
    wqlk = inp("wqlk", [128, KT, 512], BF16)      # [q0|q1|lk0|lk1] col-slices
    wlvc = inp("wlvc", [128, KT, 257], BF16)      # [lv0|lv1|Wc]
    wkv = inp("wkv", [128, KT, 256], BF16)        # [Wk|Wv] shared head
    wo = inp("wo", [128, HPC, HID], BF16)         # 0.5*Wo rows per head
    b_qlk = inp("b_qlk", [1, 512], BF16)
    b_lvc = inp("b_lvc", [1, 257], BF16)
    b_kv = inp("b_kv", [1, 256], BF16)
    tcos = inp("tcos", [128, NT, HALF], F32)      # rope cos table
    tsin = inp("tsin", [128, NT, HALF], F32)      # rope sin table
    wqk = inp("wqk", [128, 512], BF16)            # [qn_w|qn_w|kn_w|kn_w] rows
    ctA = inp("ctA", [C, HALF], F32)              # ck rope tables (block_ends)
    ctB = inp("ctB", [C, HALF], F32)
    ctC = inp("ctC", [C, HALF], F32)
    ctD = inp("ctD", [C, HALF], F32)
    ck_pass = inp("ck_pass", [C, ROPE], F32)      # kn_w[64:] bcast rows
    mbinL1 = inp("mbinL1", [128, 512], FP8)       # additive local mask [P,P,C,C]
    mbinK2 = inp("mbinK2", [128, NT, 256], FP8)   # comp mask [c, i, 2x128]
    ident8 = inp("ident8", [128, 128], FP8)       # fp8 identity
    dwide = inp("dwide", [128, 248], BF16)        # sliding block-diag pattern
    sinkkT = inp("sinkkT", [128, HPC], BF16)      # sink_k columns [d, h]
    sinkvo = inp("sinkvo", [1, HPC, 129], BF16)   # rows [sink_v[h] | 1.0]

    out_p = nc.dram_tensor("out_p", [S, HID], BF16, kind="ExternalOutput")
    dbg = {}
    if debug:
        for name, shape in [("qTb", [128, NT, 256]), ("lkTb", [128, NT, 256]),
                            ("lvo", [128, NT, HPC, 129]), ("cvo", [C, 129]),
                            ("ckT", [128, C]), ("wN", [128, NT]),
                            ("eT", [128, KT, C]),
                            ("pt5", [128, 768]), ("mg5", [128, 256]),
                            ("pstS5", [1, 256])]:
            dbg[name] = nc.dram_tensor("dbg_" + name, shape, BF16,
                                       kind="ExternalOutput")
        dbg["mgTb"] = nc.dram_tensor("dbg_mgTb", [128, HPC, S], BF16,
                                     kind="ExternalOutput")

    with tile.TileContext(nc) as tc:
        with (
            tc.tile_pool(name="const", bufs=1) as cst,
            tc.tile_pool(name="persist", bufs=1) as per,
            tc.tile_pool(name="stream", bufs=6) as stm,
            tc.tile_pool(name="scratch", bufs=3) as scr,
            tc.tile_pool(name="stats", bufs=6) as sts,
        ):
            def load(name, shape, dt):
                t = cst.tile(list(shape), dt, name=f"c_{name}")
                nc.sync.dma_start(out=t[:], in_=din[name].ap())
                return t

            # ---- weight/table loads, interleaved so P1's k-stream starts
            # immediately: each k-group (wqlk_k, wlvc_k, hT_k) arrives ~2us
            # apart and tile i=0..2 matmuls consume them as they land.
            wqlk_sb = cst.tile([128, KT, 512], BF16, name="c_wqlk")
            wlvc_sb = cst.tile([128, KT, 257], BF16, name="c_wlvc")
            hT_sb = cst.tile([128, KT, S], BF16, name="c_hT")
            nc.sync.dma_start(out=wqlk_sb[:, 0, :], in_=wqlk.ap()[:, 0, :])
            nc.sync.dma_start(out=wlvc_sb[:, 0, :], in_=wlvc.ap()[:, 0, :])
            nc.sync.dma_start(out=hT_sb[:, 0, 0:1024], in_=hT.ap()[0][:, 0:1024])
            b_qlk_sb = load("b_qlk", [1, 512], BF16)
            b_lvc_sb = load("b_lvc", [1, 257], BF16)
            tcos_sb = load("tcos", [128, NT, HALF], F32)
            tsin_sb = load("tsin", [128, NT, HALF], F32)
            wqk_sb = load("wqk", [128, 512], BF16)
            for k in range(1, KT):
                nc.sync.dma_start(out=wqlk_sb[:, k, :], in_=wqlk.ap()[:, k, :])
                nc.sync.dma_start(out=wlvc_sb[:, k, :], in_=wlvc.ap()[:, k, :])
                nc.sync.dma_start(out=hT_sb[:, k, 0:1024],
                                  in_=hT.ap()[k][:, 0:1024])
            for k in range(KT):
                nc.sync.dma_start(out=hT_sb[:, k, 1024:2048],
                                  in_=hT.ap()[k][:, 1024:2048])
            wkv_sb = load("wkv", [128, KT, 256], BF16)
            b_kv_sb = load("b_kv", [1, 256], BF16)
            ctA_sb = load("ctA", [C, HALF], F32)
            ctB_sb = load("ctB", [C, HALF], F32)
            ctC_sb = load("ctC", [C, HALF], F32)
            ctD_sb = load("ctD", [C, HALF], F32)
            ck_pass_sb = load("ck_pass", [C, ROPE], F32)
            mbinL1_sb = load("mbinL1", [128, 512], FP8)
            mbinK2_sb = load("mbinK2", [128, NT, 256], FP8)
            ident8_sb = load("ident8", [128, 128], FP8)
            dwide_sb = load("dwide", [128, 248], BF16)
            sinkkT_sb = load("sinkkT", [128, HPC], BF16)
            sinkvo_sb = load("sinkvo", [1, HPC, 129], BF16)
            wo_sb = load("wo", [128, HPC, HID], BF16)

            ident_bf = cst.tile([128, 128], BF16)
            make_identity(nc, ident_bf[:])
            ident_f32 = cst.tile([128, 128], F32)
            make_identity(nc, ident_f32[:])
            ones1 = cst.tile([1, 128], BF16)
            nc.vector.memset(ones1[:], 1.0)
            eps_t = cst.tile([128, 1], F32)
            nc.vector.memset(eps_t[:], EPS)

            # ---- persistent activations ----
            qTb = per.tile([128, NT, 256], BF16)     # q^T per head
            lkTb = per.tile([128, NT, 256], BF16)    # lk^T per head
            lvo = per.tile([128, NT, HPC, 129], BF16)  # lv | ones col
            cvo = per.tile([C, 129], BF16)           # cv | ones col
            ckT = per.tile([128, C], BF16)           # ck^T (shared head)
            mgTb = per.tile([128, HPC, S], BF16)     # merged^T
            cwN = per.tile([128, NT], F32)           # compressor scores
            wN = per.tile([128, NT], F32)            # block-softmaxed weights
            eT = per.tile([128, KT, C], BF16)        # entries^T tiles

            nc.vector.memset(lvo[:, :, :, 128:129], 1.0)
            nc.vector.memset(cvo[:, 128:129], 1.0)

            # ================= P1: projections + norm/rope =================
            with (
                tc.tile_pool(name="ps_qlk", bufs=4, space="PSUM") as pq,
                tc.tile_pool(name="ps_lvc", bufs=4, space="PSUM") as pl,
            ):
                NFILL = 4
                fill_q, fill_l = [], []
                for i in range(NFILL):
                    fq = pq.tile([128, 512], F32, tag="ps_q", name="ps_q")
                    fill_q.append(fq)
                    fl = pl.tile([128, 257], F32, tag="ps_l", name="ps_l")
                    fill_l.append(fl)
                # k-major for the first NFILL tiles: PE consumes each hT row
                # as it arrives from HBM instead of stalling on tile 0
                for k in range(KT):
                    for i in range(NFILL):
                        hk = hT_sb[:, k, i * 128:(i + 1) * 128]
                        nc.tensor.matmul(fill_q[i][:], hk, wqlk_sb[:, k, :],
                                         start=(k == 0), stop=False)
                        nc.tensor.matmul(fill_l[i][:], hk, wlvc_sb[:, k, :],
                                         start=(k == 0), stop=False)
                for i in range(NT):
                    if i < NFILL:
                        ps_q, ps_l = fill_q[i], fill_l[i]
                    else:
                        ps_q = pq.tile([128, 512], F32, tag="ps_q")
                        ps_l = pl.tile([128, 257], F32, tag="ps_l")
                        for k in range(KT):
                            hk = hT_sb[:, k, i * 128:(i + 1) * 128]
                            nc.tensor.matmul(ps_q[:], hk, wqlk_sb[:, k, :],
                                             start=(k == 0), stop=False)
                            nc.tensor.matmul(ps_l[:], hk, wlvc_sb[:, k, :],
                                             start=(k == 0), stop=False)
                    nc.tensor.matmul(ps_q[:], ones1[:], b_qlk_sb[:],
                                     start=False, stop=True)
                    nc.tensor.matmul(ps_l[:], ones1[:], b_lvc_sb[:],
                                     start=False, stop=True)

                    # rms norm (over d) for the 4 sub-tensors [q0|q1|k0|k1]
                    ssq = sts.tile([128, 4], F32)
                    for j in range(4):
                        sq_s = scr.tile([128, 128], F32, tag="sq", bufs=2)
                        nc.scalar.activation(
                            sq_s[:], ps_q[:, j * 128:(j + 1) * 128],
                            mybir.ActivationFunctionType.Square,
                            accum_out=ssq[:, j:j + 1])
                    rms = sts.tile([128, 4], F32)
                    nc.scalar.activation(rms[:], ssq[:],
                                         mybir.ActivationFunctionType.Sqrt,
                                         scale=1.0 / HD, bias=eps_t[:])
                    rinv = sts.tile([128, 4], F32)
                    nc.vector.reciprocal(rinv[:], rms[:])

                    qn = scr.tile([128, 512], BF16, tag="qn", bufs=4)
                    nc.vector.tensor_mul(
                        qn[:].rearrange("p (a b) -> p a b", a=4),
                        ps_q[:].rearrange("p (a b) -> p a b", a=4),
                        rinv[:].unsqueeze(2).broadcast_to([128, 4, 128]))
                    nc.vector.tensor_mul(qn[:], qn[:], wqk_sb[:])
                    # partial rope on cols [0:64) of each sub-tensor
                    qn4 = qn[:].rearrange("p (s r b) -> p s r b", s=2, r=2)
                    x1 = qn4[:, :, :, 0:HALF]
                    x2 = qn4[:, :, :, HALF:ROPE]

                    def tslice(t):
                        return (t[:, i, :].unsqueeze(1).unsqueeze(1)
                                .broadcast_to([128, 2, 2, HALF]))

                    t1 = scr.tile([128, 4, HALF], BF16, tag="t1", bufs=3)
                    t2 = scr.tile([128, 4, HALF], BF16, tag="t2", bufs=3)
                    t3 = scr.tile([128, 4, HALF], BF16, tag="t3", bufs=3)
                    t4 = scr.tile([128, 4, HALF], BF16, tag="t4", bufs=3)

                    def v4(t):
                        return t[:].rearrange("p (s r) c -> p s r c", s=2)

                    nc.vector.tensor_mul(v4(t1), x1, tslice(tcos_sb))
                    nc.vector.tensor_mul(v4(t2), x2, tslice(tsin_sb))
                    nc.vector.tensor_mul(v4(t3), x1, tslice(tsin_sb))
                    nc.vector.tensor_mul(v4(t4), x2, tslice(tcos_sb))
                    nc.vector.tensor_sub(x1, v4(t1), v4(t2))
                    nc.vector.tensor_add(x2, v4(t3), v4(t4))

                    # collect compressor scores + local V (with ones col)
                    nc.scalar.copy(cwN[:, i:i + 1], ps_l[:, 256:257])
                    nc.scalar.copy(
                        lvo[:, i, :, 0:128],
                        ps_l[:, 0:256].rearrange("p (h d) -> p h d", h=2))

                    # transposes into qT / lkT via DMA xbar (HWDGE is idle
                    # during P1; results not needed until P3)
                    for j in range(4):
                        dst = (qTb[:, i, j * 128:(j + 1) * 128] if j < 2 else
                               lkTb[:, i, (j - 2) * 128:(j - 1) * 128])
                        nc.sync.dma_start_transpose(
                            dst, qn[:, j * 128:(j + 1) * 128])

            # ================= P2: entries + ck/cv =================
            with (
                tc.tile_pool(name="ps_e", bufs=1, space="PSUM") as pe,
                tc.tile_pool(name="ps_kv", bufs=1, space="PSUM") as pkv,
                tc.tile_pool(name="ps_tp2", bufs=2, space="PSUM") as ptp2,
            ):
                # block-softmax of compressor scores (R=16 blocks)
                ps_cw1 = ptp2.tile([16, 128], F32, tag="tp2")
                nc.tensor.transpose(ps_cw1[:], cwN[:], ident_f32[:])
                cwT = scr.tile([16, 128], F32, tag="cwT", bufs=1)
                nc.scalar.copy(cwT[:], ps_cw1[:])
                cw3 = cwT[:].rearrange("p (g r) -> p g r", g=8)
                cmx = sts.tile([16, 8], F32)
                nc.vector.tensor_reduce(cmx[:], cw3, mybir.AxisListType.X,
                                        ALU.max)
                cwE = scr.tile([16, 128], F32, tag="cwE", bufs=1)
                nc.vector.tensor_sub(
                    cwE[:].rearrange("p (g r) -> p g r", g=8), cw3,
                    cmx[:].unsqueeze(2).broadcast_to([16, 8, 16]))
                nc.scalar.activation(cwE[:], cwE[:],
                                     mybir.ActivationFunctionType.Exp)
                csum = sts.tile([16, 8], F32)
                nc.vector.tensor_reduce(
                    csum[:], cwE[:].rearrange("p (g r) -> p g r", g=8),
                    mybir.AxisListType.X, ALU.add)
                crec = sts.tile([16, 8], F32)
                nc.vector.reciprocal(crec[:], csum[:])
                cwW = scr.tile([16, 128], F32, tag="cwW", bufs=1)
                nc.vector.tensor_mul(
                    cwW[:].rearrange("p (g r) -> p g r", g=8),
                    cwE[:].rearrange("p (g r) -> p g r", g=8),
                    crec[:].unsqueeze(2).broadcast_to([16, 8, 16]))
                ps_cw2 = ptp2.tile([128, 16], F32, tag="tp2")
                nc.tensor.transpose(ps_cw2[:], cwW[:], ident_f32[0:16, 0:16])
                nc.scalar.copy(wN[:], ps_cw2[:])

                # entries natural [C, HID] (4 bank-aligned accum groups),
                # then transpose into eT
                ps_e = pe.tile([C, HID], F32)
                entries = scr.tile([C, HID], BF16, tag="entries", bufs=1)
                for i in range(NT):
                    wbig = scr.tile([128, 128], BF16, tag="wbig", bufs=2)
                    nc.vector.tensor_scalar_mul(
                        wbig[:], dwide_sb[:, 120 - 8 * i:248 - 8 * i],
                        wN[:, i:i + 1])
                    hN_t = stm.tile([128, HID], BF16, tag="hN", bufs=5)
                    nc.sync.dma_start(out=hN_t[:], in_=hN.ap()[i])
                    for hc in range(4):
                        nc.tensor.matmul(ps_e[:, hc * 512:(hc + 1) * 512],
                                         wbig[:],
                                         hN_t[:, hc * 512:(hc + 1) * 512],
                                         start=(i == 0), stop=(i == NT - 1))
                for hc in range(4):
                    if hc % 2 == 0:
                        nc.vector.tensor_copy(entries[:, hc * 512:(hc + 1) * 512],
                                              ps_e[:, hc * 512:(hc + 1) * 512])
                    else:
                        nc.scalar.copy(entries[:, hc * 512:(hc + 1) * 512],
                                       ps_e[:, hc * 512:(hc + 1) * 512])
                for k in range(KT):
                    ps_t = ptp2.tile([128, 128], BF16, tag="tp2")
                    nc.tensor.transpose(ps_t[:],
                                        entries[:, k * 128:(k + 1) * 128],
                                        ident_bf[:])
                    if k % 2 == 0:
                        nc.vector.tensor_copy(eT[:, k, :], ps_t[:])
                    else:
                        nc.scalar.copy(eT[:, k, :], ps_t[:])

                ps_kv = pkv.tile([C, 256], F32)
                for k in range(KT):
                    nc.tensor.matmul(ps_kv[:], eT[:, k, :], wkv_sb[:, k, :],
                                     start=(k == 0), stop=False)
                nc.tensor.matmul(ps_kv[:], ones1[:], b_kv_sb[:],
                                 start=False, stop=True)

                # ck: rmsnorm + rope at block-end positions
                ssqc = sts.tile([C, 1], F32)
                sq_c = scr.tile([C, 128], F32, tag="sq", bufs=2)
                nc.scalar.activation(sq_c[:], ps_kv[:, 0:128],
                                     mybir.ActivationFunctionType.Square,
                                     accum_out=ssqc[:])
                rmsc = sts.tile([C, 1], F32)
                nc.scalar.activation(rmsc[:], ssqc[:],
                                     mybir.ActivationFunctionType.Sqrt,
                                     scale=1.0 / HD, bias=eps_t[:])
                rinvc = sts.tile([C, 1], F32)
                nc.vector.reciprocal(rinvc[:], rmsc[:])
                ckn = scr.tile([C, 128], F32, tag="ckn", bufs=1)
                nc.vector.tensor_scalar_mul(ckn[:], ps_kv[:, 0:128], rinvc[:])
                ckR = scr.tile([C, 128], BF16, tag="ckR", bufs=1)
                ct1 = scr.tile([C, HALF], F32, tag="ct1", bufs=1)
                ct2 = scr.tile([C, HALF], F32, tag="ct2", bufs=1)
                nc.vector.tensor_mul(ct1[:], ckn[:, 0:HALF], ctA_sb[:])
                nc.vector.tensor_mul(ct2[:], ckn[:, HALF:ROPE], ctB_sb[:])
                nc.vector.tensor_sub(ckR[:, 0:HALF], ct1[:], ct2[:])
                nc.vector.tensor_mul(ct1[:], ckn[:, 0:HALF], ctC_sb[:])
                nc.vector.tensor_mul(ct2[:], ckn[:, HALF:ROPE], ctD_sb[:])
                nc.vector.tensor_add(ckR[:, HALF:ROPE], ct1[:], ct2[:])
                nc.vector.tensor_mul(ckR[:, ROPE:128], ckn[:, ROPE:128],
                                     ck_pass_sb[:])
                nc.scalar.copy(cvo[:, 0:128], ps_kv[:, 128:256])
                ps_ct = ptp2.tile([128, 128], BF16, tag="tp2")
                nc.tensor.transpose(ps_ct[:], ckR[:], ident_bf[:])
                nc.scalar.copy(ckT[:], ps_ct[:])

            # ================= P3: attention (transposed scores) ===========
            with (
                tc.tile_pool(name="ps_sc", bufs=1, space="PSUM") as psc,
                tc.tile_pool(name="ps_ctx", bufs=1, space="PSUM") as pcx,
                tc.tile_pool(name="ps_snk", bufs=1, space="PSUM") as psnk,
                tc.tile_pool(name="ps_mt", bufs=1, space="PSUM") as pmt,
                tc.tile_pool(name="ps_out", bufs=2, space="PSUM") as pout,
            ):
                for i in range(NT):
                    qT_i = qTb[:, i, :]
                    # sink scores^T, heads side-by-side: [1, h*128+s]
                    ps_sink = psnk.tile([1, HPC * 128], F32)
                    for h in range(HPC):
                        nc.tensor.matmul(ps_sink[:, h * 128:(h + 1) * 128],
                                         sinkkT_sb[:, h:h + 1],
                                         qT_i[:, h * 128:(h + 1) * 128],
                                         start=True, stop=True)
                    pstS = scr.tile([1, HPC * 128], BF16, tag="pstS")
                    nc.scalar.activation(pstS[:], ps_sink[:],
                                         mybir.ActivationFunctionType.Exp,
                                         scale=SCALE)

                    # one transposed score tile [prev-h0,prev-h1,cur-h0,
                    # cur-h1,comp-h0,comp-h1], one exp, masks split
                    # Pool (local) / DVE (comp)
                    ps_s = psc.tile([128, 768], F32, tag="sc")
                    # one mask-add matmul per PSUM bank (single start=True),
                    # then score matmuls accumulate (WAW-ordered after mask)
                    nc.tensor.matmul(ps_s[:, 0:512], ident8_sb[:],
                                     mbinL1_sb[:], start=True, stop=False,
                                     skip_group_check=True)
                    nc.tensor.matmul(ps_s[:, 512:768], ident8_sb[:],
                                     mbinK2_sb[:, i, :], start=True,
                                     stop=False, skip_group_check=True)
                    for h in range(HPC):
                        qh = qT_i[:, h * 128:(h + 1) * 128]
                        if i > 0:
                            nc.tensor.matmul(
                                ps_s[:, h * 128:(h + 1) * 128],
                                lkTb[:, i - 1, h * 128:(h + 1) * 128], qh,
                                start=False, stop=True, skip_group_check=True)
                        nc.tensor.matmul(
                            ps_s[:, 256 + h * 128:256 + (h + 1) * 128],
                            lkTb[:, i, h * 128:(h + 1) * 128], qh,
                            start=False, stop=True, skip_group_check=True)
                        nc.tensor.matmul(
                            ps_s[:, 512 + h * 128:512 + (h + 1) * 128],
                            ckT[:], qh, start=False, stop=True,
                            skip_group_check=True)
                    if i == 0:
                        nc.vector.memset(ps_s[:, 0:256], -30000.0)
                    pt = scr.tile([128, 768], BF16, tag="pt", bufs=4)
                    nc.scalar.activation(pt[:], ps_s[:],
                                         mybir.ActivationFunctionType.Exp,
                                         scale=SCALE)

                    # ctx accumulation: [ctx | den] via appended ones column
                    ps_l3 = pcx.tile([128, HPC * 129], F32, tag="ctxl")
                    ps_c3 = pcx.tile([128, HPC * 129], F32, tag="ctxc")
                    for h in range(HPC):
                        o = h * 129
                        if i > 0:
                            nc.tensor.matmul(ps_l3[:, o:o + 129],
                                             pt[:, h * 128:(h + 1) * 128],
                                             lvo[:, i - 1, h, :],
                                             start=True, stop=False)
                        nc.tensor.matmul(ps_l3[:, o:o + 129],
                                         pt[:, 256 + h * 128:256 + (h + 1) * 128],
                                         lvo[:, i, h, :],
                                         start=(i == 0), stop=True)
                        nc.tensor.matmul(ps_c3[:, o:o + 129],
                                         pt[:, 512 + h * 128:512 + (h + 1) * 128],
                                         cvo[:], start=True, stop=False)
                        nc.tensor.matmul(ps_c3[:, o:o + 129],
                                         pstS[:, h * 128:(h + 1) * 128],
                                         sinkvo_sb[:, h, :],
                                         start=False, stop=True)

                    # normalize + combine: mg = ctxl*rdl + ctxc*rdc
                    rd = sts.tile([128, 4], F32)
                    nc.vector.reciprocal(
                        rd[:, 0:2].unsqueeze(2),
                        ps_l3[:].rearrange("p (h x) -> p h x", h=2)[:, :, 128:129])
                    nc.vector.reciprocal(
                        rd[:, 2:4].unsqueeze(2),
                        ps_c3[:].rearrange("p (h x) -> p h x", h=2)[:, :, 128:129])
                    mg = scr.tile([128, 256], BF16, tag="mg", bufs=2)
                    for h in range(HPC):
                        o = h * 129
                        tmp = scr.tile([128, 128], F32, tag="cmb", bufs=2)
                        nc.vector.tensor_scalar_mul(
                            tmp[:], ps_c3[:, o:o + 128], rd[:, 2 + h:3 + h])
                        nc.vector.scalar_tensor_tensor(
                            mg[:, h * 128:(h + 1) * 128],
                            ps_l3[:, o:o + 128], rd[:, h:h + 1],
                            tmp[:], ALU.mult, ALU.add)
                    if debug and i == 5:
                        nc.sync.dma_start(out=dbg["pt5"].ap(), in_=pt[:])
                        nc.sync.dma_start(out=dbg["mg5"].ap(), in_=mg[:])
                        nc.sync.dma_start(out=dbg["pstS5"].ap(), in_=pstS[:])
                    for h in range(HPC):
                        ps_mt = pmt.tile([128, 128], BF16, tag="mt")
                        nc.tensor.transpose(
                            ps_mt[:], mg[:, h * 128:(h + 1) * 128], ident_bf[:])
                        nc.vector.tensor_copy(
                            mgTb[:, h, i * 128:(i + 1) * 128], ps_mt[:])

                    # out projection, interleaved per tile
                    for oc in range(4):
                        ps_oc = pout.tile([128, 512], F32, tag="oc")
                        for h in range(HPC):
                            nc.tensor.matmul(
                                ps_oc[:], mgTb[:, h, i * 128:(i + 1) * 128],
                                wo_sb[:, h, oc * 512:(oc + 1) * 512],
                                start=(h == 0), stop=(h == HPC - 1))
                        o_sb = scr.tile([128, 512], BF16,
                                        tag=f"osb{oc % 2}", bufs=2)
                        if oc < 2:
                            nc.scalar.copy(o_sb[:], ps_oc[:])
                        else:
                            nc.vector.tensor_copy(o_sb[:], ps_oc[:])
                        nc.sync.dma_start(
                            out=out_p.ap()[i * 128:(i + 1) * 128,
                                           oc * 512:(oc + 1) * 512],
                            in_=o_sb[:])
            if debug:
                for name, t in [("qTb", qTb), ("lkTb", lkTb), ("lvo", lvo),
                                ("cvo", cvo), ("ckT", ckT), ("eT", eT),
                                ("mgTb", mgTb)]:
                    nc.sync.dma_start(out=dbg[name].ap(), in_=t[:])
                wNb = per.tile([128, NT], BF16)
                nc.vector.tensor_copy(wNb[:], wN[:])
                nc.sync.dma_start(out=dbg["wN"].ap(), in_=wNb[:])

    nc.compile()
    return nc


def _host_prep(inputs):
    """Build the 8 per-core input maps from full inputs."""
    hs = np.asarray(inputs["hidden_states"], np.float32)[0]  # [S, HID]
    Wq = np.asarray(inputs["Wq"], np.float32)
    Wc = np.asarray(inputs["Wc"], np.float32)
    Wk = np.asarray(inputs["Wk"], np.float32)
    Wv = np.asarray(inputs["Wv"], np.float32)
    Wlk = np.asarray(inputs["Wlk"], np.float32)
    Wlv = np.asarray(inputs["Wlv"], np.float32)
    qn_w = np.asarray(inputs["qn_w"], np.float32)
    kn_w = np.asarray(inputs["kn_w"], np.float32)
    sink_k = np.asarray(inputs["sink_k"], np.float32)
    sink_v = np.asarray(inputs["sink_v"], np.float32)
    Wo = np.asarray(inputs["Wo"], np.float32)
    bq = np.asarray(inputs["bq"], np.float32)
    bc = np.asarray(inputs["bc"], np.float32)
    bk = np.asarray(inputs["bk"], np.float32)
    bv = np.asarray(inputs["bv"], np.float32)
    blk = np.asarray(inputs["blk"], np.float32)
    blv = np.asarray(inputs["blv"], np.float32)

    hT_t = np.ascontiguousarray(hs.T).reshape(KT, 128, S).astype(NPBF)
    hN_t = hs.reshape(NT, 128, HID).astype(NPBF)

    def dev_w(w):  # [HID, F] -> [128, KT, F]
        return np.ascontiguousarray(
            w.reshape(KT, 128, -1).transpose(1, 0, 2)).astype(NPBF)

    # rope tables for token positions (q & k variants, norm weight folded)
    pos = np.arange(S, dtype=np.float32)
    inv_freq = 1.0 / (10000.0 ** (np.arange(HALF, dtype=np.float32) * 2.0 / ROPE))
    ang = pos[:, None] * inv_freq[None, :]
    cos, sin = np.cos(ang), np.sin(ang)  # [S, HALF]

    def pack(t):  # [S,HALF] -> [128, NT, HALF]
        return np.ascontiguousarray(
            t.reshape(NT, 128, HALF).transpose(1, 0, 2)).astype(np.float32)

    kw1, kw2 = kn_w[0:HALF], kn_w[HALF:ROPE]
    tcos = pack(cos)
    tsin = pack(sin)
    wqk = np.broadcast_to(
        np.concatenate([qn_w, qn_w, kn_w, kn_w])[None, :],
        (128, 512)).astype(NPBF).copy()

    # ck rope tables at block-end positions
    pos_c = (np.arange(C, dtype=np.float32) * R + (R - 1))
    angc = pos_c[:, None] * inv_freq[None, :]
    cosc, sinc = np.cos(angc), np.sin(angc)
    ctA = (cosc * kw1[None, :]).astype(np.float32)
    ctB = (sinc * kw2[None, :]).astype(np.float32)
    ctC = (sinc * kw1[None, :]).astype(np.float32)
    ctD = (cosc * kw2[None, :]).astype(np.float32)
    ck_pass = np.broadcast_to(kn_w[ROPE:][None, :],
                              (C, ROPE)).astype(np.float32).copy()

    # binary masks (transposed score layout [t, s] / [c, s])
    tl = np.arange(128)[:, None]
    sl = np.arange(128)[None, :]
    mbinP = np.where(tl >= sl, 0.0, -240.0).astype(np.float32)
    mbinC = np.where(tl <= sl, 0.0, -240.0).astype(np.float32)
    mbinL1 = np.concatenate([mbinP, mbinP, mbinC, mbinC], axis=1).astype(NPF8)
    cc = np.arange(128)[:, None]
    sg = np.arange(S)[None, :]
    mK = np.where(cc * R + (R - 1) <= sg, 0.0, -240.0).astype(np.float32)
    mK = mK.reshape(128, NT, 128)
    mbinK2 = np.concatenate([mK, mK], axis=2).astype(NPF8)
    ident8 = np.eye(128, dtype=np.float32).astype(NPF8)
    idx2 = np.arange(248)[None, :]
    dwide = (idx2 == 120 + np.arange(128)[:, None] // 16).astype(
        np.float32).astype(NPBF)

    common = dict(hT=hT_t, hN=hN_t, tcos=tcos, tsin=tsin, wqk=wqk,
                  ctA=ctA, ctB=ctB, ctC=ctC, ctD=ctD,
                  ck_pass=ck_pass, mbinL1=mbinL1, mbinK2=mbinK2, ident8=ident8,
                  dwide=dwide,
                  wkv=dev_w(np.concatenate([Wk, Wv], axis=1)),
                  b_kv=np.concatenate([bk, bv])[None, :].astype(NPBF))

    Wq4 = Wq.reshape(HID, NH, HD)
    Wlk4 = Wlk.reshape(HID, NH, HD)
    Wlv4 = Wlv.reshape(HID, NH, HD)
    bq4 = bq.reshape(NH, HD)
    blk4 = blk.reshape(NH, HD)
    blv4 = blv.reshape(NH, HD)
    Wo4 = Wo.reshape(NH, HD, HID)

    in_maps = []
    for c in range(NCORES):
        hh = [HPC * c + h for h in range(HPC)]
        wqlk = np.concatenate([Wq4[:, hh[0]], Wq4[:, hh[1]],
                               Wlk4[:, hh[0]], Wlk4[:, hh[1]]], axis=1)
        wlvc = np.concatenate([Wlv4[:, hh[0]], Wlv4[:, hh[1]], Wc], axis=1)
        b_qlk = np.concatenate([bq4[hh[0]], bq4[hh[1]],
                                blk4[hh[0]], blk4[hh[1]]])[None, :]
        b_lvc = np.concatenate([blv4[hh[0]], blv4[hh[1]], bc])[None, :]
        wo_c = np.ascontiguousarray(
            (0.5 * Wo4[hh]).transpose(1, 0, 2)).astype(NPBF)
        sinkkT = np.ascontiguousarray(sink_k[hh].T).astype(NPBF)  # [128, HPC]
        sinkvo = np.zeros((1, HPC, 129), np.float32)
        for h in range(HPC):
            sinkvo[0, h, 0:128] = sink_v[hh[h]]
            sinkvo[0, h, 128] = 1.0
        m = dict(common)
        m.update(wqlk=dev_w(wqlk), wlvc=dev_w(wlvc),
                 b_qlk=b_qlk.astype(NPBF), b_lvc=b_lvc.astype(NPBF),
                 wo=wo_c, sinkkT=sinkkT, sinkvo=sinkvo.astype(NPBF))
        in_maps.append(m)
    return in_maps


def kernel(**inputs):
    if "nc" not in _CACHE:
        _CACHE["nc"] = _build_bass()
    nc = _CACHE["nc"]
    in_maps = _host_prep(inputs)
    res = run_bass_kernel_spmd(nc, in_maps, core_ids=list(range(NCORES)))
    out = np.zeros((S, HID), np.float64)
    for c in range(NCORES):
        out += res.results[c]["out_p"].astype(np.float64)
    out += np.asarray(inputs["bo"], np.float32)[None, :]
    return out[None].astype(np.float32)
